# revision 11
# baseline (speedup 1.0000x reference)
"""3x3 median filter (reflect padding) on Trainium2, 8-core data parallel.

Layout (per core, 4 images):
  partition p = b*32 + g
    b in 0..3  : image index within the core's batch shard
    g in 0..31 : group of 7 consecutive output rows
  linear(p) = p*7*W*C addresses (b,g) jointly (the strides nest
  perfectly), so one 3-dim access pattern spans all 128 partitions.

All device compute is fp16 (host converts): 2-byte packed operands put
the DVE in its 2x perf mode (measured 0.553 ns/elem sustained; fp32
and u8 both run 1x) and halve DMA traffic; max quantization error
~2.5e-4 against a 2e-2 tolerance.

A single 9-row slab per partition (rows 7g-1 .. 7g+7) is loaded once
in three sub-waves on the sync + scalar HW-DGE queues: A1 = slab rows
0-1, A2 = rows 2-5, B = rows 6-8.  Each sub-wave is one contiguous
multi-row packet per partition that over-reads DRAM to absorb the
halo; the image-boundary partitions are patched by small reflect DMAs
issued on the vector queue (top) and sync queue (bottom) so the first
compute op waits only on A1 + the top patches.  The A2 wave reaches
slab row 5, which makes the entire row 0-3 pipeline (stage 1, merge,
final med3, store) independent of wave B.

Median of 9 = med3( max3(col_lows), med3(col_meds), min3(col_highs) )
with each vertical column triple sorted once and shared across the
three horizontally adjacent windows.  The vertical sort additionally
shares the row-pair min/max between the two triples that contain the
pair (triple k = pair(k,k+1) + row k+2 for even k, row k + pair
(k+1,k+2)... realized as: triple k uses pair j = k for even k, j =
k+1 for odd k), cutting stage-1 volume from 6N to 5.14N at the price
of even/odd split combine ops (stride-2 row access patterns keep the
DVE in 2x mode, which only requires a packed last dim).

Horizontal neighbor access is a +-3 float shift inside each row; the
image's first/last output columns are recomputed exactly with narrow
per-column ops and overwritten before the store.  The final med3
chain + store go out in three row groups so stores overlap the tail
compute.
"""

import sys

if "/opt/trn_rl_repo" not in sys.path:
    sys.path.insert(0, "/opt/trn_rl_repo")

import numpy as np

import concourse.bass as bass  # noqa: F401
import concourse.tile as tile
from concourse import bacc, mybir
from concourse.ap import AP
from concourse.bass_utils import run_bass_kernel_spmd

F32 = mybir.dt.float32
F16 = mybir.dt.float16
MIN = mybir.AluOpType.min
MAX = mybir.AluOpType.max

B, H, W, C = 32, 224, 224, 3
NCORES = 8
BPC = B // NCORES      # 4 images per core
NG, GR = 32, 7         # row-groups per image, rows per group
WC = W * C             # 672 floats per image row
IMG = H * WC
PS = GR * WC           # 4704: per-partition linear stride
R = GR                 # 7 output rows per partition
N = R * WC             # 4704 output floats per partition
SRR = R + 2            # 9 slab rows

_CACHE = {}


def _build_kernel(tc, y, x):
    nc = tc.nc
    qa, qb, qv = nc.sync, nc.scalar, nc.gpsimd

    with tc.tile_pool(name="sb", bufs=1) as sb:
        S = sb.tile([128, SRR, WC], F16, tag="s", name="S")

        def rows(q, p0, p1, dram_row, s0, nr):
            q.dma_start(S[p0:p1, s0:s0 + nr, :],
                        AP(x.tensor, p0 * PS + dram_row * WC,
                           [[PS, p1 - p0], [1, nr * WC]]))

        # ---- loads ------------------------------------------------
        # Every piece writes a disjoint region of S (waves split at
        # the image-boundary partitions), so no DMA-DMA WAW ordering
        # exists anywhere and all pieces issue immediately.
        # reflect top (slab row 0 at p = 0,32,64,96 <- image row 1)
        qa.dma_start(S[0:128:32, 0:1, :],
                     AP(x.tensor, WC, [[IMG, 4], [1, WC]]))
        # slab row 1 at the same partitions (<- image row 0)
        qb.dma_start(S[0:128:32, 1:2, :],
                     AP(x.tensor, 0, [[IMG, 4], [1, WC]]))
        # wave A1: slab rows 0..1 (dram -1..0 over-read), skipping the
        # image-boundary partitions
        rows(qa, 1, 32, -1, 0, 2)
        rows(qa, 33, 64, -1, 0, 2)
        rows(qb, 65, 96, -1, 0, 2)
        rows(qb, 97, 128, -1, 0, 2)
        # wave A2: slab rows 2..5 (dram 1..4), all partitions
        rows(qa, 0, 64, 1, 2, 4)
        rows(qb, 64, 128, 1, 2, 4)
        # wave B: slab rows 6..8 (dram 5..7 over-read), skipping the
        # bottom-boundary partitions p = 31,63,95,127
        rows(qa, 0, 31, 5, 6, 3)
        rows(qa, 32, 63, 5, 6, 3)
        rows(qb, 64, 95, 5, 6, 3)
        rows(qb, 96, 127, 5, 6, 3)
        # slab rows 6..7 at p = 31,63,95,127
        qb.dma_start(S[31:128:32, 6:8, :],
                     AP(x.tensor, 31 * PS + 5 * WC, [[32 * PS, 4], [1, 2 * WC]]))
        # reflect bottom (slab row 8 at p = 31,63,95,127 <- image row
        # 222): gpsimd SWDGE is laggy (~9us issue-to-data) but this is
        # only needed by the k=4,6 combines ~19us in
        qv.dma_start(S[31:128:32, 8:9, :],
                     AP(x.tensor, (H - 2) * WC, [[IMG, 4], [1, WC]]))

        # ---- stage 1: vertical column sort with pair sharing -------
        # Pp/Qp row j2 holds min/max of slab rows (2*j2, 2*j2+1).
        # Triple k (slab rows k..k+2) uses pair j = k (k even, third
        # row below) or j = k+1 (k odd, third row above).
        Pp = sb.tile([128, 4, WC], F16, tag="pp", name="Pp")
        Qp = sb.tile([128, 4, WC], F16, tag="qp", name="Qp")
        LO = sb.tile([128, R, WC], F16, tag="lo", name="LO")
        ME = sb.tile([128, R, WC], F16, tag="me", name="ME")
        HI = sb.tile([128, R, WC], F16, tag="hi", name="HI")
        T1 = sb.tile([128, R, WC], F16, tag="t1", name="T1")

        # pairs j=0 (needs A1 + top patches only)
        nc.vector.tensor_tensor(Pp[:, 0:1], S[:, 0:1], S[:, 1:2], MIN)
        nc.vector.tensor_tensor(Qp[:, 0:1], S[:, 0:1], S[:, 1:2], MAX)
        # pairs j=2,4 (needs A2)
        nc.vector.tensor_tensor(Pp[:, 1:3], S[:, 2:6:2], S[:, 3:6:2], MIN)
        nc.vector.tensor_tensor(Qp[:, 1:3], S[:, 2:6:2], S[:, 3:6:2], MAX)

        def combine(lo, me, hi, t1, pp, qp, c):
            nc.vector.tensor_tensor(lo, pp, c, MIN)
            nc.vector.tensor_tensor(t1, qp, c, MIN)
            nc.vector.tensor_tensor(hi, qp, c, MAX)
            nc.vector.tensor_tensor(me, pp, t1, MAX)

        # combines k=0,2 (pairs 0,1; c = S rows 2,4)
        combine(LO[:, 0:3:2], ME[:, 0:3:2], HI[:, 0:3:2], T1[:, 0:3:2],
                Pp[:, 0:2], Qp[:, 0:2], S[:, 2:5:2])
        # combines k=1,3 (pairs 1,2; c = S rows 1,3)
        combine(LO[:, 1:4:2], ME[:, 1:4:2], HI[:, 1:4:2], T1[:, 1:4:2],
                Pp[:, 1:3], Qp[:, 1:3], S[:, 1:4:2])

        M1 = sb.tile([128, R, WC], F16, tag="m1", name="M1")

        # ---- stage 2 declarations (bodies emitted in stream order) -
        E = WC - 3   # 669
        D = WC - 6   # 666
        U = sb.tile([128, R, WC], F16, tag="u", name="U")
        V = sb.tile([128, R, WC], F16, tag="v", name="V")
        Sm = sb.tile([128, R, WC], F16, tag="sm", name="Sm")
        Tm = sb.tile([128, R, WC], F16, tag="tm", name="Tm")
        MT = sb.tile([128, R, WC], F16, tag="mt", name="MT")
        A = U   # max3 of lows
        Cc = V  # min3 of highs
        Bm = Sm  # med3 of meds

        def merge(ra, rb):
            nc.vector.tensor_tensor(U[:, ra:rb, 0:E], LO[:, ra:rb, 0:E],
                                    LO[:, ra:rb, 3:WC], MAX)
            nc.vector.tensor_tensor(U[:, ra:rb, 0:D], U[:, ra:rb, 0:D],
                                    LO[:, ra:rb, 6:WC], MAX)
            nc.vector.tensor_tensor(V[:, ra:rb, 0:E], HI[:, ra:rb, 0:E],
                                    HI[:, ra:rb, 3:WC], MIN)
            nc.vector.tensor_tensor(V[:, ra:rb, 0:D], V[:, ra:rb, 0:D],
                                    HI[:, ra:rb, 6:WC], MIN)
            nc.vector.tensor_tensor(Sm[:, ra:rb, 0:E], ME[:, ra:rb, 0:E],
                                    ME[:, ra:rb, 3:WC], MIN)
            nc.vector.tensor_tensor(Tm[:, ra:rb, 0:E], ME[:, ra:rb, 0:E],
                                    ME[:, ra:rb, 3:WC], MAX)
            nc.vector.tensor_tensor(Tm[:, ra:rb, 0:D], Tm[:, ra:rb, 0:D],
                                    ME[:, ra:rb, 6:WC], MIN)
            nc.vector.tensor_tensor(Sm[:, ra:rb, 0:D], Sm[:, ra:rb, 0:D],
                                    Tm[:, ra:rb, 0:D], MAX)

        def final_compute(ra, rb):
            nc.vector.tensor_tensor(MT[:, ra:rb, 0:D], A[:, ra:rb, 0:D],
                                    Bm[:, ra:rb, 0:D], MIN)
            nc.vector.tensor_tensor(A[:, ra:rb, 0:D], A[:, ra:rb, 0:D],
                                    Bm[:, ra:rb, 0:D], MAX)
            nc.vector.tensor_tensor(Cc[:, ra:rb, 0:D], A[:, ra:rb, 0:D],
                                    Cc[:, ra:rb, 0:D], MIN)
            nc.vector.tensor_tensor(M1[:, ra:rb, 3:WC - 3],
                                    MT[:, ra:rb, 0:D],
                                    Cc[:, ra:rb, 0:D], MAX)

        def store(ra, rb):
            for (p0, p1, q) in ((0, 64, qa), (64, 128, qb)):
                dst = AP(y.tensor, p0 * PS + ra * WC,
                         [[PS, p1 - p0], [WC, rb - ra], [1, WC]])
                q.dma_start(dst, M1[p0:p1, ra:rb, :])

        # pairs j=6 (needs B)
        nc.vector.tensor_tensor(Pp[:, 3:4], S[:, 6:7], S[:, 7:8], MIN)
        nc.vector.tensor_tensor(Qp[:, 3:4], S[:, 6:7], S[:, 7:8], MAX)
        # combines k=5 (pair 3; c = S row 5 -- no dependency on the
        # late bottom patch, so it runs while that patch lands)
        combine(LO[:, 5:6], ME[:, 5:6], HI[:, 5:6], T1[:, 5:6],
                Pp[:, 3:4], Qp[:, 3:4], S[:, 5:6])
        # combines k=4,6 (pairs 2,3; c = S rows 6,8)
        combine(LO[:, 4:7:2], ME[:, 4:7:2], HI[:, 4:7:2], T1[:, 4:7:2],
                Pp[:, 2:4], Qp[:, 2:4], S[:, 6:9:2])

        # ---- exact first/last output columns (reflect), both at once
        # col 0: window cols (1,0,1) -> med3(max(lo0,lo1), med1,
        # min(hi0,hi1)); col 223: window cols (222,223,222).
        L4 = LO.rearrange("p r (a c) -> p r a c", a=W, c=C)
        H4 = HI.rearrange("p r (a c) -> p r a c", a=W, c=C)
        T4 = ME.rearrange("p r (a c) -> p r a c", a=W, c=C)
        M4 = M1.rearrange("p r (a c) -> p r a c", a=W, c=C)
        lo_o = L4[:, :, 0:W:W - 1, :]      # cols {0, 223}
        lo_i = L4[:, :, 1:W:W - 3, :]      # cols {1, 222}
        hi_o = H4[:, :, 0:W:W - 1, :]
        hi_i = H4[:, :, 1:W:W - 3, :]
        be = T4[:, :, 1:W:W - 3, :]        # med of inner col
        ae = sb.tile([128, R, 2, C], F16, tag="ae", name="ae")
        ce = sb.tile([128, R, 2, C], F16, tag="ce", name="ce")
        mem = sb.tile([128, R, 2, C], F16, tag="mm", name="mm")
        nc.vector.tensor_tensor(ae[:], lo_o, lo_i, MAX)
        nc.vector.tensor_tensor(ce[:], hi_o, hi_i, MIN)
        nc.vector.tensor_tensor(mem[:], ae[:], be, MIN)
        nc.vector.tensor_tensor(ae[:], ae[:], be, MAX)
        nc.vector.tensor_tensor(ce[:], ae[:], ce[:], MIN)
        nc.vector.tensor_tensor(M4[:, :, 0:W:W - 1, :], mem[:], ce[:], MAX)

        # ---- tail: single merge pass over all rows (wave B lands
        # well before stage-1 A-work drains, so no stall), then the
        # final med3 in three groups with a 1-row last group so the
        # last store is minimal
        merge(0, 7)
        final_compute(0, 4)
        store(0, 4)
        final_compute(4, 6)
        store(4, 6)
        final_compute(6, 7)
        store(6, 7)


def _build():
    if "nc" in _CACHE:
        return _CACHE["nc"]
    nc = bacc.Bacc("TRN2", target_bir_lowering=False, debug=False)
    x = nc.dram_tensor("x", [BPC, H, W, C], F16, kind="ExternalInput").ap()
    y = nc.dram_tensor("y", [BPC, H, W, C], F16, kind="ExternalOutput").ap()
    with tile.TileContext(nc) as tc:
        _build_kernel(tc, y, x)
    nc.compile()
    _CACHE["nc"] = nc
    return nc


def run(input_batch, **spmd_kwargs):
    nc = _build()
    xh = np.ascontiguousarray(input_batch).astype(np.float16)
    in_maps = [
        {"x": np.ascontiguousarray(xh[i * BPC:(i + 1) * BPC])}
        for i in range(NCORES)
    ]
    res = run_bass_kernel_spmd(nc, in_maps, list(range(NCORES)), **spmd_kwargs)
    out = np.concatenate([r["y"] for r in res.results],
                         axis=0).astype(np.float32)
    return out, res


def kernel(input_batch):
    out, _ = run(np.asarray(input_batch))
    return out


# revision 16
# speedup vs baseline: 1.2372x; 1.2372x over previous
"""3x3 median filter (reflect padding) on Trainium2, 8-core data parallel.

Layout (per core, 4 images):
  partition p = b*32 + g
    b in 0..3  : image index within the core's batch shard
    g in 0..31 : group of 7 consecutive output rows
  linear(p) = p*7*W*C addresses (b,g) jointly (the strides nest
  perfectly), so one 3-dim access pattern spans all 128 partitions.

All device compute is fp16 (host converts): 2-byte packed operands put
the DVE in its 2x perf mode (measured 0.553 ns/elem sustained; fp32
and u8 both run 1x) and halve DMA traffic; max quantization error
~2.5e-4 against a 2e-2 tolerance.

A single 9-row slab per partition (rows 7g-1 .. 7g+7) is loaded once
in three sub-waves on the sync + scalar HW-DGE queues: A1 = slab rows
0-1, A2 = rows 2-5, B = rows 6-8.  Each sub-wave is one contiguous
multi-row packet per partition that over-reads DRAM to absorb the
halo; the image-boundary partitions are patched by small reflect DMAs
issued on the vector queue (top) and sync queue (bottom) so the first
compute op waits only on A1 + the top patches.  The A2 wave reaches
slab row 5, which makes the entire row 0-3 pipeline (stage 1, merge,
final med3, store) independent of wave B.

Median of 9 = med3( max3(col_lows), med3(col_meds), min3(col_highs) )
with each vertical column triple sorted once and shared across the
three horizontally adjacent windows.  The vertical sort additionally
shares the row-pair min/max between the two triples that contain the
pair (triple k = pair(k,k+1) + row k+2 for even k, row k + pair
(k+1,k+2)... realized as: triple k uses pair j = k for even k, j =
k+1 for odd k), cutting stage-1 volume from 6N to 5.14N at the price
of even/odd split combine ops (stride-2 row access patterns keep the
DVE in 2x mode, which only requires a packed last dim).

Horizontal neighbor access is a +-3 float shift inside each row; the
image's first/last output columns are recomputed exactly with narrow
per-column ops and overwritten before the store.  The final med3
chain + store go out in three row groups so stores overlap the tail
compute.
"""

import sys

if "/opt/trn_rl_repo" not in sys.path:
    sys.path.insert(0, "/opt/trn_rl_repo")

import numpy as np

import concourse.bass as bass  # noqa: F401
import concourse.tile as tile
from concourse import bacc, mybir
from concourse.ap import AP
from concourse.bass_utils import run_bass_kernel_spmd

F32 = mybir.dt.float32
F16 = mybir.dt.float16
MIN = mybir.AluOpType.min
MAX = mybir.AluOpType.max

B, H, W, C = 32, 224, 224, 3
NCORES = 8
BPC = B // NCORES      # 4 images per core
NG, GR = 32, 7         # row-groups per image, rows per group
WC = W * C             # 672 floats per image row
IMG = H * WC
PS = GR * WC           # 4704: per-partition linear stride
R = GR                 # 7 output rows per partition
N = R * WC             # 4704 output floats per partition
SRR = R + 2            # 9 slab rows

_CACHE = {}


def _build_kernel(tc, y, x):
    nc = tc.nc
    qa, qb, qv = nc.sync, nc.scalar, nc.gpsimd

    with tc.tile_pool(name="sb", bufs=1) as sb:
        S = sb.tile([128, SRR, WC], F16, tag="s", name="S")

        def rows(q, p0, p1, dram_row, s0, nr):
            q.dma_start(S[p0:p1, s0:s0 + nr, :],
                        AP(x.tensor, p0 * PS + dram_row * WC,
                           [[PS, p1 - p0], [1, nr * WC]]))

        # ---- loads ------------------------------------------------
        # DMA pieces must span ~64 partitions to spread across the 16
        # DMA engines (narrow pieces serialize onto one engine, ~13x
        # slower).  Wave A1 carries slab rows 1..2 (dram rows 0..1):
        # in-bounds for every partition, no over-read, no reflect
        # involvement -- so the first compute op (row pair (1,2))
        # waits on nothing but A1.  The reflect rows 0/8 for the
        # image-boundary partitions are fixed by tiny single-partition
        # DVE copies (reflect identity: S0 = S2, S8 = S6) off the
        # critical path.
        # wave A1: slab rows 1..2 (dram 0..1)
        rows(qa, 0, 64, 0, 1, 2)
        rows(qb, 64, 128, 0, 1, 2)
        # wave A2: slab rows 3..6 (dram 2..5)
        rows(qa, 0, 64, 2, 3, 4)
        rows(qb, 64, 128, 2, 3, 4)
        # slab row 0 (dram -1) for p >= 1; boundary partitions get
        # garbage here, overwritten by the S0 = S2 copies below
        rows(qa, 1, 64, -1, 0, 1)
        rows(qb, 64, 128, -1, 0, 1)
        # wave B: slab rows 7..8 (dram 6..7, over-read at the bottom
        # boundaries; fixed by the S8 = S6 copies below)
        rows(qa, 0, 64, 6, 7, 2)
        rows(qb, 64, 127, 6, 7, 2)
        qb.dma_start(S[127:128, 7:8, :],     # p127 slab row 7
                     AP(x.tensor, 127 * PS + 6 * WC, [[1, WC]]))
        # reflect bottom (slab row 8 at p = 31,63,95,127 <- image row
        # 222): DVE copies can't start at partition 31 (quadrant
        # rule), so this stays a DMA patch; its WAW-wait on wave B
        # resolves ~17us, well before the k=6 combine needs it
        qa.dma_start(S[31:128:32, 8:9, :],
                     AP(x.tensor, (H - 2) * WC, [[IMG, 4], [1, WC]]))

        # ---- stage 1: vertical column sort with pair sharing -------
        # Pp/Qp row m holds min/max of slab rows (2m+1, 2m+2).
        # Triple k (slab rows k..k+2) uses pair m = k/2 with c = S[k]
        # (k even) or m = (k-1)/2 with c = S[k+2] (k odd).
        Pp = sb.tile([128, 4, WC], F16, tag="pp", name="Pp")
        Qp = sb.tile([128, 4, WC], F16, tag="qp", name="Qp")
        LO = sb.tile([128, R, WC], F16, tag="lo", name="LO")
        ME = sb.tile([128, R, WC], F16, tag="me", name="ME")
        HI = sb.tile([128, R, WC], F16, tag="hi", name="HI")
        T1 = sb.tile([128, R, WC], F16, tag="t1", name="T1")

        # pairs m=0 (slab rows 1,2: wave A1 only -- the first op)
        nc.vector.tensor_tensor(Pp[:, 0:1], S[:, 1:2], S[:, 2:3], MIN)
        nc.vector.tensor_tensor(Qp[:, 0:1], S[:, 1:2], S[:, 2:3], MAX)
        # pairs m=1,2 (slab rows 3..6: wave A2)
        nc.vector.tensor_tensor(Pp[:, 1:3], S[:, 3:6:2], S[:, 4:7:2], MIN)
        nc.vector.tensor_tensor(Qp[:, 1:3], S[:, 3:6:2], S[:, 4:7:2], MAX)

        # reflect top: S0 = S2 at the image-boundary partitions
        # (overwrites the dram-row -1 garbage; S2 is wave A1)
        for p in (0, 32, 64, 96):
            nc.vector.tensor_tensor(S[p:p + 1, 0:1], S[p:p + 1, 2:3],
                                    S[p:p + 1, 2:3], MIN)

        def combine(lo, me, hi, t1, pp, qp, c):
            nc.vector.tensor_tensor(lo, pp, c, MIN)
            nc.vector.tensor_tensor(t1, qp, c, MIN)
            nc.vector.tensor_tensor(hi, qp, c, MAX)
            nc.vector.tensor_tensor(me, pp, t1, MAX)

        # combines k=0,2,4 (pairs m=0,1,2; c = S rows 0,2,4)
        combine(LO[:, 0:5:2], ME[:, 0:5:2], HI[:, 0:5:2], T1[:, 0:5:2],
                Pp[:, 0:3], Qp[:, 0:3], S[:, 0:5:2])
        # combines k=1,3 (pairs m=0,1; c = S rows 3,5)
        combine(LO[:, 1:4:2], ME[:, 1:4:2], HI[:, 1:4:2], T1[:, 1:4:2],
                Pp[:, 0:2], Qp[:, 0:2], S[:, 3:6:2])

        M1 = sb.tile([128, R, WC], F16, tag="m1", name="M1")

        # ---- stage 2 declarations (bodies emitted in stream order) -
        E = WC - 3   # 669
        D = WC - 6   # 666
        U = sb.tile([128, R, WC], F16, tag="u", name="U")
        V = sb.tile([128, R, WC], F16, tag="v", name="V")
        Sm = sb.tile([128, R, WC], F16, tag="sm", name="Sm")
        Tm = sb.tile([128, R, WC], F16, tag="tm", name="Tm")
        MT = sb.tile([128, R, WC], F16, tag="mt", name="MT")
        A = U   # max3 of lows
        Cc = V  # min3 of highs
        Bm = Sm  # med3 of meds

        def merge(ra, rb):
            nc.vector.tensor_tensor(U[:, ra:rb, 0:E], LO[:, ra:rb, 0:E],
                                    LO[:, ra:rb, 3:WC], MAX)
            nc.vector.tensor_tensor(U[:, ra:rb, 0:D], U[:, ra:rb, 0:D],
                                    LO[:, ra:rb, 6:WC], MAX)
            nc.vector.tensor_tensor(V[:, ra:rb, 0:E], HI[:, ra:rb, 0:E],
                                    HI[:, ra:rb, 3:WC], MIN)
            nc.vector.tensor_tensor(V[:, ra:rb, 0:D], V[:, ra:rb, 0:D],
                                    HI[:, ra:rb, 6:WC], MIN)
            nc.vector.tensor_tensor(Sm[:, ra:rb, 0:E], ME[:, ra:rb, 0:E],
                                    ME[:, ra:rb, 3:WC], MIN)
            nc.vector.tensor_tensor(Tm[:, ra:rb, 0:E], ME[:, ra:rb, 0:E],
                                    ME[:, ra:rb, 3:WC], MAX)
            nc.vector.tensor_tensor(Tm[:, ra:rb, 0:D], Tm[:, ra:rb, 0:D],
                                    ME[:, ra:rb, 6:WC], MIN)
            nc.vector.tensor_tensor(Sm[:, ra:rb, 0:D], Sm[:, ra:rb, 0:D],
                                    Tm[:, ra:rb, 0:D], MAX)

        def final_compute(ra, rb):
            nc.vector.tensor_tensor(MT[:, ra:rb, 0:D], A[:, ra:rb, 0:D],
                                    Bm[:, ra:rb, 0:D], MIN)
            nc.vector.tensor_tensor(A[:, ra:rb, 0:D], A[:, ra:rb, 0:D],
                                    Bm[:, ra:rb, 0:D], MAX)
            nc.vector.tensor_tensor(Cc[:, ra:rb, 0:D], A[:, ra:rb, 0:D],
                                    Cc[:, ra:rb, 0:D], MIN)
            nc.vector.tensor_tensor(M1[:, ra:rb, 3:WC - 3],
                                    MT[:, ra:rb, 0:D],
                                    Cc[:, ra:rb, 0:D], MAX)

        def store(ra, rb):
            for (p0, p1, q) in ((0, 64, qa), (64, 128, qb)):
                dst = AP(y.tensor, p0 * PS + ra * WC,
                         [[PS, p1 - p0], [WC, rb - ra], [1, WC]])
                q.dma_start(dst, M1[p0:p1, ra:rb, :])

        # pairs m=3 (slab rows 7,8: wave B + the bottom patch)
        nc.vector.tensor_tensor(Pp[:, 3:4], S[:, 7:8], S[:, 8:9], MIN)
        nc.vector.tensor_tensor(Qp[:, 3:4], S[:, 7:8], S[:, 8:9], MAX)
        # combines k=5 (pair m=2; c = S row 7: wave B)
        combine(LO[:, 5:6], ME[:, 5:6], HI[:, 5:6], T1[:, 5:6],
                Pp[:, 2:3], Qp[:, 2:3], S[:, 7:8])
        # combines k=6 (pair m=3; c = S row 6)
        combine(LO[:, 6:7], ME[:, 6:7], HI[:, 6:7], T1[:, 6:7],
                Pp[:, 3:4], Qp[:, 3:4], S[:, 6:7])

        # ---- exact first/last output columns (reflect), both at once
        # col 0: window cols (1,0,1) -> med3(max(lo0,lo1), med1,
        # min(hi0,hi1)); col 223: window cols (222,223,222).
        L4 = LO.rearrange("p r (a c) -> p r a c", a=W, c=C)
        H4 = HI.rearrange("p r (a c) -> p r a c", a=W, c=C)
        T4 = ME.rearrange("p r (a c) -> p r a c", a=W, c=C)
        M4 = M1.rearrange("p r (a c) -> p r a c", a=W, c=C)
        lo_o = L4[:, :, 0:W:W - 1, :]      # cols {0, 223}
        lo_i = L4[:, :, 1:W:W - 3, :]      # cols {1, 222}
        hi_o = H4[:, :, 0:W:W - 1, :]
        hi_i = H4[:, :, 1:W:W - 3, :]
        be = T4[:, :, 1:W:W - 3, :]        # med of inner col
        ae = sb.tile([128, R, 2, C], F16, tag="ae", name="ae")
        ce = sb.tile([128, R, 2, C], F16, tag="ce", name="ce")
        mem = sb.tile([128, R, 2, C], F16, tag="mm", name="mm")
        nc.vector.tensor_tensor(ae[:], lo_o, lo_i, MAX)
        nc.vector.tensor_tensor(ce[:], hi_o, hi_i, MIN)
        nc.vector.tensor_tensor(mem[:], ae[:], be, MIN)
        nc.vector.tensor_tensor(ae[:], ae[:], be, MAX)
        nc.vector.tensor_tensor(ce[:], ae[:], ce[:], MIN)
        nc.vector.tensor_tensor(M4[:, :, 0:W:W - 1, :], mem[:], ce[:], MAX)

        # ---- tail: single merge pass over all rows (wave B lands
        # well before stage-1 A-work drains, so no stall), then the
        # final med3 in three groups with a 1-row last group so the
        # last store is minimal
        merge(0, 7)
        final_compute(0, 4)
        store(0, 4)
        final_compute(4, 6)
        store(4, 6)
        final_compute(6, 7)
        store(6, 7)


def _build():
    if "nc" in _CACHE:
        return _CACHE["nc"]
    nc = bacc.Bacc("TRN2", target_bir_lowering=False, debug=False)
    x = nc.dram_tensor("x", [BPC, H, W, C], F16, kind="ExternalInput").ap()
    y = nc.dram_tensor("y", [BPC, H, W, C], F16, kind="ExternalOutput").ap()
    with tile.TileContext(nc) as tc:
        _build_kernel(tc, y, x)
    nc.compile()
    _CACHE["nc"] = nc
    return nc


def run(input_batch, **spmd_kwargs):
    nc = _build()
    xh = np.ascontiguousarray(input_batch).astype(np.float16)
    in_maps = [
        {"x": np.ascontiguousarray(xh[i * BPC:(i + 1) * BPC])}
        for i in range(NCORES)
    ]
    res = run_bass_kernel_spmd(nc, in_maps, list(range(NCORES)), **spmd_kwargs)
    out = np.concatenate([r["y"] for r in res.results],
                         axis=0).astype(np.float32)
    return out, res


def kernel(input_batch):
    out, _ = run(np.asarray(input_batch))
    return out


# revision 17
# speedup vs baseline: 1.4596x; 1.1797x over previous
"""3x3 median filter (reflect padding) on Trainium2, 8-core data parallel.

Layout (per core, 4 images):
  partition p = b*32 + g
    b in 0..3  : image index within the core's batch shard
    g in 0..31 : group of 7 consecutive output rows
  linear(p) = p*7*W*C addresses (b,g) jointly (the strides nest
  perfectly), so one 3-dim access pattern spans all 128 partitions.

All device compute is fp16 (host converts): 2-byte packed operands put
the DVE in its 2x perf mode (measured 0.553 ns/elem sustained; fp32
and u8 both run 1x).  Multi-row (3D) access patterns cost ~95ns per
row segment on the DVE, so every big op is FLAT (single segment
spanning rows); the merge stage's +-3 shifts then bleed across row
boundaries, but only into columns >= 666 of each row, which no
consumer reads (the final med3 consumes cols 0..665 and the image's
first/last output columns are recomputed exactly by the edge block).

Loads: DMA pieces must span ~64 partitions to spread across the 16
DMA engines (narrow pieces serialize onto ONE engine, ~13x slower).
Wave A1 carries slab rows 1..3 (dram 0..2): in-bounds everywhere, no
over-read, no reflect involvement, so the first compute op waits on
nothing but A1.  A2 = slab rows 4..6, then the 1-row slab-row-0 piece
(dram -1, boundary partitions get garbage, fixed by single-partition
DVE copies S0 = S2 which are legal because the image-boundary
partitions 0,32,64,96 sit on DVE partition-quadrant starts), then
wave B = slab rows 7..8 (over-read at the bottom boundaries, fixed by
a reflect patch DMA whose WAW-wait on wave B resolves off the
critical path; a DVE copy is illegal there since p=31,... are not
quadrant starts).

Median of 9 = med3( max3(col_lows), med3(col_meds), min3(col_highs) )
with each vertical column triple sorted once (P/Q pair min/max then
lo/med/hi, 6N flat ops) and shared across the three horizontally
adjacent windows.  The final med3 chain + store go out in three row
groups so stores overlap the tail compute.
"""

import sys

if "/opt/trn_rl_repo" not in sys.path:
    sys.path.insert(0, "/opt/trn_rl_repo")

import numpy as np

import concourse.bass as bass  # noqa: F401
import concourse.tile as tile
from concourse import bacc, mybir
from concourse.ap import AP
from concourse.bass_utils import run_bass_kernel_spmd

F32 = mybir.dt.float32
F16 = mybir.dt.float16
MIN = mybir.AluOpType.min
MAX = mybir.AluOpType.max

B, H, W, C = 32, 224, 224, 3
NCORES = 8
BPC = B // NCORES      # 4 images per core
NG, GR = 32, 7         # row-groups per image, rows per group
WC = W * C             # 672 floats per image row
IMG = H * WC
PS = GR * WC           # 4704: per-partition linear stride
R = GR                 # 7 output rows per partition
N = R * WC             # 4704 output floats per partition
SRR = R + 2            # 9 slab rows

_CACHE = {}


def _build_kernel(tc, y, x):
    nc = tc.nc
    qa, qb = nc.sync, nc.scalar

    with tc.tile_pool(name="sb", bufs=1) as sb:
        S = sb.tile([128, SRR, WC], F16, tag="s", name="S")

        def rows(q, p0, p1, dram_row, s0, nr):
            q.dma_start(S[p0:p1, s0:s0 + nr, :],
                        AP(x.tensor, p0 * PS + dram_row * WC,
                           [[PS, p1 - p0], [1, nr * WC]]))

        # ---- loads ------------------------------------------------
        # wave A1: slab rows 1..3 (dram 0..2)
        rows(qa, 0, 64, 0, 1, 3)
        rows(qb, 64, 128, 0, 1, 3)
        # wave A2: slab rows 4..6 (dram 3..5)
        rows(qa, 0, 64, 3, 4, 3)
        rows(qb, 64, 128, 3, 4, 3)
        # slab row 0 (dram -1) for p >= 1; boundary partitions get
        # garbage here, overwritten by the S0 = S2 copies below
        rows(qa, 1, 64, -1, 0, 1)
        rows(qb, 64, 128, -1, 0, 1)
        # wave B: slab rows 7..8 (dram 6..7, over-read at the bottom
        # boundaries)
        rows(qa, 0, 64, 6, 7, 2)
        rows(qb, 64, 127, 6, 7, 2)
        qb.dma_start(S[127:128, 7:8, :],     # p127 slab row 7
                     AP(x.tensor, 127 * PS + 6 * WC, [[1, WC]]))
        # reflect bottom (slab row 8 at p = 31,63,95,127 <- image row
        # 222); WAW-wait on wave B resolves well before pairs m3
        qa.dma_start(S[31:128:32, 8:9, :],
                     AP(x.tensor, (H - 2) * WC, [[IMG, 4], [1, WC]]))

        Sf = S.rearrange("p r f -> p (r f)")

        # ---- stage 1: vertical column sort (flat ops) --------------
        # P/Q[k] = min/max(S[k], S[k+1]); LO/ME/HI[k] = sorted triple
        # (k, k+1, k+2), range-split to chase the arriving waves.
        P = sb.tile([128, N], F16, tag="p", name="P")
        Q = sb.tile([128, N], F16, tag="q", name="Q")
        LO = sb.tile([128, R, WC], F16, tag="lo", name="LO")
        ME = sb.tile([128, R, WC], F16, tag="me", name="ME")
        HI = sb.tile([128, R, WC], F16, tag="hi", name="HI")
        T1 = sb.tile([128, R, WC], F16, tag="t1", name="T1")
        LOf = LO.rearrange("p r f -> p (r f)")
        MEf = ME.rearrange("p r f -> p (r f)")
        HIf = HI.rearrange("p r f -> p (r f)")
        T1f = T1.rearrange("p r f -> p (r f)")

        def s1_pq(fa, fb):
            nc.vector.tensor_tensor(P[:, fa:fb], Sf[:, fa:fb],
                                    Sf[:, fa + WC:fb + WC], MIN)
            nc.vector.tensor_tensor(Q[:, fa:fb], Sf[:, fa:fb],
                                    Sf[:, fa + WC:fb + WC], MAX)

        def s1_cols(fa, fb):
            nc.vector.tensor_tensor(LOf[:, fa:fb], P[:, fa:fb],
                                    Sf[:, fa + 2 * WC:fb + 2 * WC], MIN)
            nc.vector.tensor_tensor(T1f[:, fa:fb], Q[:, fa:fb],
                                    Sf[:, fa + 2 * WC:fb + 2 * WC], MIN)
            nc.vector.tensor_tensor(HIf[:, fa:fb], Q[:, fa:fb],
                                    Sf[:, fa + 2 * WC:fb + 2 * WC], MAX)
            nc.vector.tensor_tensor(MEf[:, fa:fb], P[:, fa:fb],
                                    T1f[:, fa:fb], MAX)

        # A1 (slab rows 1..3): pairs (1,2), (2,3)
        s1_pq(WC, 3 * WC)
        # triple at output row 1 (slab 1,2,3) -- A1 only
        s1_cols(WC, 2 * WC)
        # reflect top: S0 = S2 at the image-boundary partitions
        # (overwrites the dram-row -1 garbage; legal DVE ops since
        # 0,32,64,96 are partition-quadrant starts)
        for p in (0, 32, 64, 96):
            nc.vector.tensor_tensor(S[p:p + 1, 0:1], S[p:p + 1, 2:3],
                                    S[p:p + 1, 2:3], MIN)
        # A2 (slab rows 4..6): pairs (3,4),(4,5),(5,6)
        s1_pq(3 * WC, 6 * WC)
        # pair (0,1) -- needs slab row 0 (+ copies)
        s1_pq(0, WC)
        # triples at output rows 0, 2..4 (slab rows <= 6)
        s1_cols(0, WC)
        s1_cols(2 * WC, 5 * WC)
        # B (slab rows 7..8): pair (6,7)
        s1_pq(6 * WC, N)
        # triples at output rows 5..6
        s1_cols(5 * WC, N)

        M1 = sb.tile([128, R, WC], F16, tag="m1", name="M1")

        # ---- exact first/last output columns (reflect), both at once
        # col 0: window cols (1,0,1) -> med3(max(lo0,lo1), med1,
        # min(hi0,hi1)); col 223: window cols (222,223,222).
        L4 = LO.rearrange("p r (a c) -> p r a c", a=W, c=C)
        H4 = HI.rearrange("p r (a c) -> p r a c", a=W, c=C)
        T4 = ME.rearrange("p r (a c) -> p r a c", a=W, c=C)
        M4 = M1.rearrange("p r (a c) -> p r a c", a=W, c=C)
        lo_o = L4[:, :, 0:W:W - 1, :]      # cols {0, 223}
        lo_i = L4[:, :, 1:W:W - 3, :]      # cols {1, 222}
        hi_o = H4[:, :, 0:W:W - 1, :]
        hi_i = H4[:, :, 1:W:W - 3, :]
        be = T4[:, :, 1:W:W - 3, :]        # med of inner col
        ae = sb.tile([128, R, 2, C], F16, tag="ae", name="ae")
        ce = sb.tile([128, R, 2, C], F16, tag="ce", name="ce")
        mem = sb.tile([128, R, 2, C], F16, tag="mm", name="mm")
        nc.vector.tensor_tensor(ae[:], lo_o, lo_i, MAX)
        nc.vector.tensor_tensor(ce[:], hi_o, hi_i, MIN)
        nc.vector.tensor_tensor(mem[:], ae[:], be, MIN)
        nc.vector.tensor_tensor(ae[:], ae[:], be, MAX)
        nc.vector.tensor_tensor(ce[:], ae[:], ce[:], MIN)
        nc.vector.tensor_tensor(M4[:, :, 0:W:W - 1, :], mem[:], ce[:], MAX)

        # ---- stage 2: horizontal merge, all FLAT single-segment ops
        # over the whole 7-row slab; the +-3/-6 shifts bleed across
        # row boundaries but only into per-row columns >= 666, which
        # nothing consumes.
        E = N - 3
        D = N - 6
        U = sb.tile([128, R, WC], F16, tag="u", name="U")
        V = sb.tile([128, R, WC], F16, tag="v", name="V")
        Sm = sb.tile([128, R, WC], F16, tag="sm", name="Sm")
        Tm = sb.tile([128, R, WC], F16, tag="tm", name="Tm")
        MT = sb.tile([128, R, WC], F16, tag="mt", name="MT")
        Uf = U.rearrange("p r f -> p (r f)")
        Vf = V.rearrange("p r f -> p (r f)")
        Smf = Sm.rearrange("p r f -> p (r f)")
        Tmf = Tm.rearrange("p r f -> p (r f)")
        MTf = MT.rearrange("p r f -> p (r f)")

        nc.vector.tensor_tensor(Uf[:, 0:E], LOf[:, 0:E], LOf[:, 3:N], MAX)
        nc.vector.tensor_tensor(Uf[:, 0:D], Uf[:, 0:D], LOf[:, 6:N], MAX)
        nc.vector.tensor_tensor(Vf[:, 0:E], HIf[:, 0:E], HIf[:, 3:N], MIN)
        nc.vector.tensor_tensor(Vf[:, 0:D], Vf[:, 0:D], HIf[:, 6:N], MIN)
        nc.vector.tensor_tensor(Smf[:, 0:E], MEf[:, 0:E], MEf[:, 3:N], MIN)
        nc.vector.tensor_tensor(Tmf[:, 0:E], MEf[:, 0:E], MEf[:, 3:N], MAX)
        nc.vector.tensor_tensor(Tmf[:, 0:D], Tmf[:, 0:D], MEf[:, 6:N], MIN)
        nc.vector.tensor_tensor(Smf[:, 0:D], Smf[:, 0:D], Tmf[:, 0:D], MAX)

        A = Uf   # max3 of lows
        Cc = Vf  # min3 of highs
        Bm = Smf  # med3 of meds

        # ---- final med3 chain + store in 3 row groups (3D output
        # APs: no cross-row garbage may touch the edge columns that
        # the edge block already wrote)
        def final(ra, rb):
            nc.vector.tensor_tensor(MT[:, ra:rb, 0:WC - 6],
                                    U[:, ra:rb, 0:WC - 6],
                                    Sm[:, ra:rb, 0:WC - 6], MIN)
            nc.vector.tensor_tensor(U[:, ra:rb, 0:WC - 6],
                                    U[:, ra:rb, 0:WC - 6],
                                    Sm[:, ra:rb, 0:WC - 6], MAX)
            nc.vector.tensor_tensor(V[:, ra:rb, 0:WC - 6],
                                    U[:, ra:rb, 0:WC - 6],
                                    V[:, ra:rb, 0:WC - 6], MIN)
            nc.vector.tensor_tensor(M1[:, ra:rb, 3:WC - 3],
                                    MT[:, ra:rb, 0:WC - 6],
                                    V[:, ra:rb, 0:WC - 6], MAX)
            for (p0, p1, q) in ((0, 64, qa), (64, 128, qb)):
                dst = AP(y.tensor, p0 * PS + ra * WC,
                         [[PS, p1 - p0], [WC, rb - ra], [1, WC]])
                q.dma_start(dst, M1[p0:p1, ra:rb, :])

        final(0, 3)
        final(3, 5)
        final(5, 7)


def _build():
    if "nc" in _CACHE:
        return _CACHE["nc"]
    nc = bacc.Bacc("TRN2", target_bir_lowering=False, debug=False)
    x = nc.dram_tensor("x", [BPC, H, W, C], F16, kind="ExternalInput").ap()
    y = nc.dram_tensor("y", [BPC, H, W, C], F16, kind="ExternalOutput").ap()
    with tile.TileContext(nc) as tc:
        _build_kernel(tc, y, x)
    nc.compile()
    _CACHE["nc"] = nc
    return nc


def run(input_batch, **spmd_kwargs):
    nc = _build()
    xh = np.ascontiguousarray(input_batch).astype(np.float16)
    in_maps = [
        {"x": np.ascontiguousarray(xh[i * BPC:(i + 1) * BPC])}
        for i in range(NCORES)
    ]
    res = run_bass_kernel_spmd(nc, in_maps, list(range(NCORES)), **spmd_kwargs)
    out = np.concatenate([r["y"] for r in res.results],
                         axis=0).astype(np.float32)
    return out, res


def kernel(input_batch):
    out, _ = run(np.asarray(input_batch))
    return out


# revision 19
# speedup vs baseline: 1.5062x; 1.0319x over previous
"""3x3 median filter (reflect padding) on Trainium2, 8-core data parallel.

Layout (per core, 4 images):
  partition p = b*32 + g
    b in 0..3  : image index within the core's batch shard
    g in 0..31 : group of 7 consecutive output rows
  linear(p) = p*7*W*C addresses (b,g) jointly (the strides nest
  perfectly), so one 3-dim access pattern spans all 128 partitions.

All device compute is fp16 (host converts): 2-byte packed operands put
the DVE in its 2x perf mode (measured 0.553 ns/elem sustained; fp32
and u8 both run 1x).  Multi-row (3D) access patterns cost ~95ns per
row segment on the DVE, so every big op is FLAT (single segment
spanning rows); the merge stage's +-3 shifts then bleed across row
boundaries, but only into columns >= 666 of each row, which no
consumer reads (the final med3 consumes cols 0..665 and the image's
first/last output columns are recomputed exactly by the edge block).

Loads: DMA pieces must span ~64 partitions to spread across the 16
DMA engines (narrow pieces serialize onto ONE engine, ~13x slower).
Wave A1 carries slab rows 1..3 (dram 0..2): in-bounds everywhere, no
over-read, no reflect involvement, so the first compute op waits on
nothing but A1.  A2 = slab rows 4..6, then the 1-row slab-row-0 piece
(dram -1, boundary partitions get garbage, fixed by single-partition
DVE copies S0 = S2 which are legal because the image-boundary
partitions 0,32,64,96 sit on DVE partition-quadrant starts), then
wave B = slab rows 7..8 (over-read at the bottom boundaries, fixed by
a reflect patch DMA whose WAW-wait on wave B resolves off the
critical path; a DVE copy is illegal there since p=31,... are not
quadrant starts).

Median of 9 = med3( max3(col_lows), med3(col_meds), min3(col_highs) )
with each vertical column triple sorted once (P/Q pair min/max then
lo/med/hi, 6N flat ops) and shared across the three horizontally
adjacent windows.  The final med3 chain + store go out in three row
groups so stores overlap the tail compute.
"""

import sys

if "/opt/trn_rl_repo" not in sys.path:
    sys.path.insert(0, "/opt/trn_rl_repo")

import numpy as np

import concourse.bass as bass  # noqa: F401
import concourse.tile as tile
from concourse import bacc, mybir
from concourse.ap import AP
from concourse.bass_utils import run_bass_kernel_spmd

F32 = mybir.dt.float32
F16 = mybir.dt.float16
MIN = mybir.AluOpType.min
MAX = mybir.AluOpType.max

B, H, W, C = 32, 224, 224, 3
NCORES = 8
BPC = B // NCORES      # 4 images per core
NG, GR = 32, 7         # row-groups per image, rows per group
WC = W * C             # 672 floats per image row
IMG = H * WC
PS = GR * WC           # 4704: per-partition linear stride
R = GR                 # 7 output rows per partition
N = R * WC             # 4704 output floats per partition
SRR = R + 2            # 9 slab rows

_CACHE = {}


def _build_kernel(tc, y, x):
    nc = tc.nc
    qa, qb = nc.sync, nc.scalar

    with tc.tile_pool(name="sb", bufs=1) as sb:
        S = sb.tile([128, SRR, WC], F16, tag="s", name="S")

        def rows(q, p0, p1, dram_row, s0, nr):
            q.dma_start(S[p0:p1, s0:s0 + nr, :],
                        AP(x.tensor, p0 * PS + dram_row * WC,
                           [[PS, p1 - p0], [1, nr * WC]]))

        # ---- loads ------------------------------------------------
        # wave A1: slab rows 1..3 (dram 0..2)
        rows(qa, 0, 64, 0, 1, 3)
        rows(qb, 64, 128, 0, 1, 3)
        # wave A2: slab rows 4..6 (dram 3..5)
        rows(qa, 0, 64, 3, 4, 3)
        rows(qb, 64, 128, 3, 4, 3)
        # slab row 0 (dram -1) for p >= 1; boundary partitions get
        # garbage here, fixed by the reflect top patch below
        rows(qa, 1, 64, -1, 0, 1)
        rows(qb, 64, 128, -1, 0, 1)
        # reflect top (slab row 0 at p = 0,32,64,96 <- image row 1);
        # WAW-wait on the row-0 pieces resolves ~17us, well before the
        # (0,1) pair ops need it (~26us)
        qb.dma_start(S[0:128:32, 0:1, :],
                     AP(x.tensor, WC, [[IMG, 4], [1, WC]]))
        # wave B: slab rows 7..8 (dram 6..7, over-read at the bottom
        # boundaries)
        rows(qa, 0, 64, 6, 7, 2)
        rows(qb, 64, 127, 6, 7, 2)
        qb.dma_start(S[127:128, 7:8, :],     # p127 slab row 7
                     AP(x.tensor, 127 * PS + 6 * WC, [[1, WC]]))
        # reflect bottom (slab row 8 at p = 31,63,95,127 <- image row
        # 222); WAW-wait on wave B resolves well before pairs m3
        qa.dma_start(S[31:128:32, 8:9, :],
                     AP(x.tensor, (H - 2) * WC, [[IMG, 4], [1, WC]]))

        Sf = S.rearrange("p r f -> p (r f)")

        # ---- stage 1: vertical column sort (flat ops) --------------
        # P/Q[k] = min/max(S[k], S[k+1]); LO/ME/HI[k] = sorted triple
        # (k, k+1, k+2), range-split to chase the arriving waves.
        P = sb.tile([128, N], F16, tag="p", name="P")
        Q = sb.tile([128, N], F16, tag="q", name="Q")
        LO = sb.tile([128, R, WC], F16, tag="lo", name="LO")
        ME = sb.tile([128, R, WC], F16, tag="me", name="ME")
        HI = sb.tile([128, R, WC], F16, tag="hi", name="HI")
        T1 = sb.tile([128, R, WC], F16, tag="t1", name="T1")
        LOf = LO.rearrange("p r f -> p (r f)")
        MEf = ME.rearrange("p r f -> p (r f)")
        HIf = HI.rearrange("p r f -> p (r f)")
        T1f = T1.rearrange("p r f -> p (r f)")

        def s1_pq(fa, fb):
            nc.vector.tensor_tensor(P[:, fa:fb], Sf[:, fa:fb],
                                    Sf[:, fa + WC:fb + WC], MIN)
            nc.vector.tensor_tensor(Q[:, fa:fb], Sf[:, fa:fb],
                                    Sf[:, fa + WC:fb + WC], MAX)

        def s1_cols(fa, fb):
            nc.vector.tensor_tensor(LOf[:, fa:fb], P[:, fa:fb],
                                    Sf[:, fa + 2 * WC:fb + 2 * WC], MIN)
            nc.vector.tensor_tensor(T1f[:, fa:fb], Q[:, fa:fb],
                                    Sf[:, fa + 2 * WC:fb + 2 * WC], MIN)
            nc.vector.tensor_tensor(HIf[:, fa:fb], Q[:, fa:fb],
                                    Sf[:, fa + 2 * WC:fb + 2 * WC], MAX)
            nc.vector.tensor_tensor(MEf[:, fa:fb], P[:, fa:fb],
                                    T1f[:, fa:fb], MAX)

        # A1 (slab rows 1..3): pairs (1,2), (2,3)
        s1_pq(WC, 3 * WC)
        # triple at output row 1 (slab 1,2,3) -- A1 only
        s1_cols(WC, 2 * WC)
        # A2 (slab rows 4..6): pairs (3,4),(4,5),(5,6)
        s1_pq(3 * WC, 6 * WC)
        # pair (0,1) -- needs slab row 0 (+ copies)
        s1_pq(0, WC)
        # triples at output rows 0, 2..4 (slab rows <= 6)
        s1_cols(0, WC)
        s1_cols(2 * WC, 5 * WC)
        # B (slab rows 7..8): pair (6,7)
        s1_pq(6 * WC, N)
        # triples at output rows 5..6
        s1_cols(5 * WC, N)

        M1 = sb.tile([128, R, WC], F16, tag="m1", name="M1")

        # ---- exact first/last output columns (reflect), both at once
        # col 0: window cols (1,0,1) -> med3(max(lo0,lo1), med1,
        # min(hi0,hi1)); col 223: window cols (222,223,222).
        L4 = LO.rearrange("p r (a c) -> p r a c", a=W, c=C)
        H4 = HI.rearrange("p r (a c) -> p r a c", a=W, c=C)
        T4 = ME.rearrange("p r (a c) -> p r a c", a=W, c=C)
        M4 = M1.rearrange("p r (a c) -> p r a c", a=W, c=C)
        lo_o = L4[:, :, 0:W:W - 1, :]      # cols {0, 223}
        lo_i = L4[:, :, 1:W:W - 3, :]      # cols {1, 222}
        hi_o = H4[:, :, 0:W:W - 1, :]
        hi_i = H4[:, :, 1:W:W - 3, :]
        be = T4[:, :, 1:W:W - 3, :]        # med of inner col
        ae = sb.tile([128, R, 2, C], F16, tag="ae", name="ae")
        ce = sb.tile([128, R, 2, C], F16, tag="ce", name="ce")
        mem = sb.tile([128, R, 2, C], F16, tag="mm", name="mm")
        nc.vector.tensor_tensor(ae[:], lo_o, lo_i, MAX)
        nc.vector.tensor_tensor(ce[:], hi_o, hi_i, MIN)
        nc.vector.tensor_tensor(mem[:], ae[:], be, MIN)
        nc.vector.tensor_tensor(ae[:], ae[:], be, MAX)
        nc.vector.tensor_tensor(ce[:], ae[:], ce[:], MIN)
        nc.vector.tensor_tensor(M4[:, :, 0:W:W - 1, :], mem[:], ce[:], MAX)

        # ---- stage 2: horizontal merge, all FLAT single-segment ops
        # over the whole 7-row slab; the +-3/-6 shifts bleed across
        # row boundaries but only into per-row columns >= 666, which
        # nothing consumes.
        E = N - 3
        D = N - 6
        U = sb.tile([128, R, WC], F16, tag="u", name="U")
        V = sb.tile([128, R, WC], F16, tag="v", name="V")
        Sm = sb.tile([128, R, WC], F16, tag="sm", name="Sm")
        Tm = sb.tile([128, R, WC], F16, tag="tm", name="Tm")
        MT = sb.tile([128, R, WC], F16, tag="mt", name="MT")
        Uf = U.rearrange("p r f -> p (r f)")
        Vf = V.rearrange("p r f -> p (r f)")
        Smf = Sm.rearrange("p r f -> p (r f)")
        Tmf = Tm.rearrange("p r f -> p (r f)")
        MTf = MT.rearrange("p r f -> p (r f)")

        nc.vector.tensor_tensor(Uf[:, 0:E], LOf[:, 0:E], LOf[:, 3:N], MAX)
        nc.vector.tensor_tensor(Uf[:, 0:D], Uf[:, 0:D], LOf[:, 6:N], MAX)
        nc.vector.tensor_tensor(Vf[:, 0:E], HIf[:, 0:E], HIf[:, 3:N], MIN)
        nc.vector.tensor_tensor(Vf[:, 0:D], Vf[:, 0:D], HIf[:, 6:N], MIN)
        nc.vector.tensor_tensor(Smf[:, 0:E], MEf[:, 0:E], MEf[:, 3:N], MIN)
        nc.vector.tensor_tensor(Tmf[:, 0:E], MEf[:, 0:E], MEf[:, 3:N], MAX)
        nc.vector.tensor_tensor(Tmf[:, 0:D], Tmf[:, 0:D], MEf[:, 6:N], MIN)
        nc.vector.tensor_tensor(Smf[:, 0:D], Smf[:, 0:D], Tmf[:, 0:D], MAX)

        A = Uf   # max3 of lows
        Cc = Vf  # min3 of highs
        Bm = Smf  # med3 of meds

        # ---- final med3 chain + store in 3 row groups (3D output
        # APs: no cross-row garbage may touch the edge columns that
        # the edge block already wrote)
        def final(ra, rb):
            nc.vector.tensor_tensor(MT[:, ra:rb, 0:WC - 6],
                                    U[:, ra:rb, 0:WC - 6],
                                    Sm[:, ra:rb, 0:WC - 6], MIN)
            nc.vector.tensor_tensor(U[:, ra:rb, 0:WC - 6],
                                    U[:, ra:rb, 0:WC - 6],
                                    Sm[:, ra:rb, 0:WC - 6], MAX)
            nc.vector.tensor_tensor(V[:, ra:rb, 0:WC - 6],
                                    U[:, ra:rb, 0:WC - 6],
                                    V[:, ra:rb, 0:WC - 6], MIN)
            nc.vector.tensor_tensor(M1[:, ra:rb, 3:WC - 3],
                                    MT[:, ra:rb, 0:WC - 6],
                                    V[:, ra:rb, 0:WC - 6], MAX)
            for (p0, p1, q) in ((0, 64, qa), (64, 128, qb)):
                dst = AP(y.tensor, p0 * PS + ra * WC,
                         [[PS, p1 - p0], [WC, rb - ra], [1, WC]])
                q.dma_start(dst, M1[p0:p1, ra:rb, :])

        final(0, 3)
        final(3, 5)
        final(5, 7)


def _build():
    if "nc" in _CACHE:
        return _CACHE["nc"]
    nc = bacc.Bacc("TRN2", target_bir_lowering=False, debug=False)
    x = nc.dram_tensor("x", [BPC, H, W, C], F16, kind="ExternalInput").ap()
    y = nc.dram_tensor("y", [BPC, H, W, C], F16, kind="ExternalOutput").ap()
    with tile.TileContext(nc) as tc:
        _build_kernel(tc, y, x)
    nc.compile()
    _CACHE["nc"] = nc
    return nc


def run(input_batch, **spmd_kwargs):
    nc = _build()
    xh = np.ascontiguousarray(input_batch).astype(np.float16)
    in_maps = [
        {"x": np.ascontiguousarray(xh[i * BPC:(i + 1) * BPC])}
        for i in range(NCORES)
    ]
    res = run_bass_kernel_spmd(nc, in_maps, list(range(NCORES)), **spmd_kwargs)
    out = np.concatenate([r["y"] for r in res.results],
                         axis=0).astype(np.float32)
    return out, res


def kernel(input_batch):
    out, _ = run(np.asarray(input_batch))
    return out
